# revision 1
# baseline (speedup 1.0000x reference)
"""Tied-row (MSA) attention on 8 Trainium2 NeuronCores.

Reference computation (B=128, n=512, dim=256, h=8, dh=64,
r=tie_attn_dim=64, b=B//r=2):
    q = x @ Wq ; k,v = split(x @ Wkv)
    dots[b,h,i,j] = sum_{r,d} q[b,r,h,i,d] k[b,r,h,j,d] * scale
    attn = softmax_j(dots)
    out[b,r,h,i,d] = sum_j attn[b,h,i,j] v[b,r,h,j,d]
    y = out @ Wo + bo

Sharding: 8 cores = b(2) x head-pairs(4).  Each core owns one batch
element and 2 of the 8 heads and produces the partial
    y_part = out[:, :, own 2 heads, :] @ Wo[own 128 rows, :]
The host sums the 4 partials per b and adds bo (the head reduction of
the output projection commutes with the sum).

Per-core device kernel (shapes hardcoded):
  inputs : xT [64, 256, 512] f16   (x[b] transposed to [r, c, n])
           wq,wk,wv [256, 128] f16 (wq pre-scaled by dh^-.5 * r^-.5)
           wo [128, 256] f16
  output : y  [64, 512, 256] f32   (partial)

  Phase 1 + dots wave A fused (r-loop): qT_r/kT_r projections -> PSUM ->
          resident fp16 q_all/k_all [128=(2h x 64d), r, n]; dots for
          i-tiles 0,1 accumulate in 4 banks one iteration behind the
          copies (PSUM: 2 q + 2 k + 4 dots = 8 banks; the two heads'
          K=64 dots matmuls auto row-tile via base_partition 0/64 and
          run concurrently).  Wave A softmax inside this PSUM scope.
  Wave B: dots i-tiles 2,3 accumulate from resident q/k; attn tiles are
          transposed to attnT fp16 by single xbar DMA transposes
          (out[j, jc, i] = attn[i, jc*128+j]), overlapping the wave.
  Phase 3 (r-loop, 2-deep SW pipeline A=v, B=out, C=y): reload xT_r,
          v_r = xT_r.T @ wv, out_rT[hd, i] over j-chunks (lhsT = v f16,
          rhs = attnT f16; jc-outer/h-inner emission so the two heads'
          M=64 matmuls col-tile concurrently, skip_group_check for the
          interleaved PSUM groups), y_r[i, e] = out_rT.T @ wo, 8-row
          blocked DMA out on the ACT queue.

  Built with bacc.Bacc(): its compile() pass legalizes Tile's sync for
  this walrus (which caps sync waits per instruction); callers must
  finalize() the program before running (see _get_program).
"""

import os
import sys

for _p in ("/opt/trn_rl_repo", "/root/.axon_site/_ro/trn_rl_repo"):
    if os.path.isdir(_p) and _p not in sys.path:
        sys.path.insert(0, _p)

import numpy as np

R = 64          # tie dim (MSA rows per batch element)
RB = 8          # rows per DMA block
N = 512         # sequence length
C = 256         # model dim
HP = 128        # head-pair width: 2 heads x 64
E = 256         # output dim
NCORES = 8

_CACHE = {}


def build_program(phases=(1, 2, 3)):
    import concourse.bacc as bacc
    from concourse import mybir
    from concourse.tile import TileContext
    from contextlib import ExitStack

    f32 = mybir.dt.float32
    f16 = mybir.dt.float16

    # Bacc (not bass.Bass): its compile() pass legalizes sync for walrus --
    # moves matmul waits onto LDWEIGHTS and lowers multi-wait instructions
    # to event semaphores.  Raw Tile output violates walrus's per-struct
    # sync-wait limits.
    nc = bacc.Bacc()
    xT = nc.declare_dram_parameter("xT", [R, C, N], f16, isOutput=False)
    wq = nc.declare_dram_parameter("wq", [C, HP], f16, isOutput=False)
    wk = nc.declare_dram_parameter("wk", [C, HP], f16, isOutput=False)
    wv = nc.declare_dram_parameter("wv", [C, HP], f16, isOutput=False)
    wo = nc.declare_dram_parameter("wo", [HP, E], f16, isOutput=False)
    y = nc.declare_dram_parameter("y", [R, N, E], f32, isOutput=True)

    # xT block rb viewed as [p, r_in_block, c_chunk, n]
    xT_blk = xT.rearrange("(rb r) (cc p) n -> rb p r cc n", r=RB, p=128)
    # y block rb viewed as [p, r_in_block, i_tile, e]
    y_blk = y.rearrange("(rb r) (t p) e -> rb p r t e", r=RB, p=128)

    with TileContext(nc) as tc, ExitStack() as ctx:
        singles = ctx.enter_context(tc.tile_pool(name="singles", bufs=1))
        sm = ctx.enter_context(tc.tile_pool(name="sm", bufs=4))
        attnp = ctx.enter_context(tc.tile_pool(name="attnp", bufs=4))
        attntp = ctx.enter_context(tc.tile_pool(name="attntp", bufs=2))

        # weights: [256, X] -> sbuf [128, 2, X] (c-chunk on free axis)
        wq_sb = singles.tile([128, 2, HP], f16)
        wk_sb = singles.tile([128, 2, HP], f16)
        wv_sb = singles.tile([128, 2, HP], f16)
        wo_sb = singles.tile([128, E], f16)
        for cc in range(2):
            nc.gpsimd.dma_start(out=wq_sb[:, cc, :], in_=wq[cc * 128:(cc + 1) * 128, :])
            nc.gpsimd.dma_start(out=wk_sb[:, cc, :], in_=wk[cc * 128:(cc + 1) * 128, :])
            nc.gpsimd.dma_start(out=wv_sb[:, cc, :], in_=wv[cc * 128:(cc + 1) * 128, :])
        nc.gpsimd.dma_start(out=wo_sb, in_=wo[:, :])

        # attnT survives into phase 3: kernel-scoped pool
        attnT = [attntp.tile([128, 4, N], f16, tag="attnT", name=f"attnT_{h}")
                 for h in range(2)]

        def softmax(dots_hit, h, it):
            """dots PSUM tile -> normalized f16 attn SBUF tile.

            No max-subtraction: dots = q k^T with the 1/(sqrt(dh) sqrt(r))
            scale folded into Wq, so entries are ~N(0,1) and exp cannot
            overflow fp32/fp16.  This keeps ACT as the only dots reader
            (walrus allows at most 2 sync waits per instruction)."""
            ssum = sm.tile([128, 1], f32, tag="ssum", bufs=8)
            rinv = sm.tile([128, 1], f32, tag="rinv", bufs=8)
            attn = attnp.tile([128, N], f16, tag="attn", bufs=8,
                              name=f"attn_{h}_{it}")
            nc.scalar.activation(
                out=attn, in_=dots_hit,
                func=mybir.ActivationFunctionType.Exp,
                accum_out=ssum)
            nc.vector.reciprocal(rinv, ssum)
            nc.vector.tensor_scalar_mul(attn, attn, rinv)
            return attn

        def transpose_attn(ps_pool, attn, h, it):
            # one f16 xbar DMA transpose, SBUF -> SBUF: out[j, jc, i] =
            # attn[i, jc*128 + j]; no PE/PSUM involvement
            nc.sync.dma_start_transpose(
                out=attnT[h][:, :, it * 128:(it + 1) * 128], in_=attn)

        xpool = ctx.enter_context(tc.tile_pool(name="xpool", bufs=2))

        # resident fp16 qT/kT live only through phases 1-2
        with tc.tile_pool(name="resid", bufs=1) as resid:
            q_all = resid.tile([128, R, N], f16)
            k_all = resid.tile([128, R, N], f16)

            def dots_wave(dots_tiles, r, its):
                for it in its:
                    for h in range(2):
                        hs = slice(h * 64, (h + 1) * 64)
                        nc.tensor.matmul(
                            dots_tiles[h][it % 2],
                            lhsT=q_all[hs, r, it * 128:(it + 1) * 128],
                            rhs=k_all[hs, r, :],
                            start=(r == 0), stop=(r == R - 1))

            # -------- Phase 1 + dots wave A (i-tiles 0,1) fused --------
            attnA = {}
            with tc.tile_pool(name="ps1", space="PSUM", bufs=2) as ps1:
                dotsA = [[ps1.tile([128, N], f32, tag="dots", bufs=4,
                                   name=f"dotsA_{h}_{it}")
                          for it in range(2)] for h in range(2)]
                n_r = R if 1 in phases else 0
                for r in range(n_r + 1):
                    if r < n_r:
                        rb, ri = divmod(r, RB)
                        if ri == 0:
                            x_sb = xpool.tile([128, RB, 2, N], f16, tag="x",
                                              name=f"x1_{rb}")
                            nc.sync.dma_start(out=x_sb, in_=xT_blk[rb])
                        q_ps = ps1.tile([128, N], f32, tag="q")
                        k_ps = ps1.tile([128, N], f32, tag="k")
                        for cc in range(2):
                            nc.tensor.matmul(q_ps, lhsT=wq_sb[:, cc, :],
                                             rhs=x_sb[:, ri, cc, :],
                                             start=(cc == 0), stop=(cc == 1))
                        for cc in range(2):
                            nc.tensor.matmul(k_ps, lhsT=wk_sb[:, cc, :],
                                             rhs=x_sb[:, ri, cc, :],
                                             start=(cc == 0), stop=(cc == 1))
                        nc.vector.tensor_copy(q_all[:, r, :], q_ps)
                        nc.scalar.copy(k_all[:, r, :], k_ps)
                    if 0 <= r - 1 < n_r and 2 in phases:
                        dots_wave(dotsA, r - 1, (0, 1))
                # wave A softmax consumes the dots PSUM inside this scope
                for h in range(2 if 2 in phases else 0):
                    for it in range(2):
                        attnA[(h, it)] = softmax(dotsA[h][it], h, it)

            # -------- dots wave B + all transposes --------
            with tc.tile_pool(name="ps2", space="PSUM", bufs=2) as ps2:
                dotsB = [[ps2.tile([128, N], f32, tag="dots", bufs=4,
                                   name=f"dotsB_{h}_{it}")
                          for it in range(2)] for h in range(2)]
                for r in range(R if 2 in phases else 0):
                    dots_wave(dotsB, r, (2, 3))
                # wave A transposes overlap wave B's accumulation (PE is
                # in-order, but DVE copies and softmaxes interleave)
                for (h, it), attn in attnA.items():
                    transpose_attn(ps2, attn, h, it)
                for h in range(2 if 2 in phases else 0):
                    for it in (2, 3):
                        attn = softmax(dotsB[h][it % 2], h, it)
                        transpose_attn(ps2, attn, h, it)

        # ---------------- Phase 3: v, out, y (2-deep SW pipeline) ----------------
        with tc.tile_pool(name="ps3", space="PSUM", bufs=2) as ps3, \
             tc.tile_pool(name="vpool", bufs=4) as vpool, \
             tc.tile_pool(name="outp", bufs=4) as outp, \
             tc.tile_pool(name="ypool", bufs=2) as ypool:
            n_r = R if 3 in phases else 0
            v_sbs = {}
            out_sbs = {}
            y_sbs = {}

            def stage_a(r, x_sb, ri):
                v_ps = ps3.tile([128, 4, 128], f32, tag="v", name=f"v_ps_{r}")
                for jt in range(4):
                    for cc in range(2):
                        nc.tensor.matmul(
                            v_ps[:, jt, :],
                            lhsT=x_sb[:, ri, cc, jt * 128:(jt + 1) * 128],
                            rhs=wv_sb[:, cc, :],
                            start=(cc == 0), stop=(cc == 1))
                v_sb = vpool.tile([128, 4, 128], f16, tag="vsb", name=f"v_sb_{r}")
                nc.scalar.copy(v_sb, v_ps)
                v_sbs[r] = v_sb

            def stage_b(r):
                v_sb = v_sbs.pop(r)
                out_ps = ps3.tile([128, N], f32, tag="out", name=f"out_ps_{r}")
                # jc-outer / h-inner: adjacent matmuls hit different PE col
                # groups (out partitions 0-63 / 64-127) and run concurrently
                for jc in range(4):
                    for h in range(2):
                        hs = slice(h * 64, (h + 1) * 64)
                        nc.tensor.matmul(
                            out_ps[hs, :],
                            lhsT=v_sb[:, jc, hs],
                            rhs=attnT[h][:, jc, :],
                            start=(jc == 0), stop=(jc == 3),
                            skip_group_check=True)
                out_sb = outp.tile([128, N], f16, tag="outsb", name=f"out_sb_{r}")
                nc.vector.tensor_copy(out_sb, out_ps)
                out_sbs[r] = out_sb

            def stage_c(r):
                out_sb = out_sbs.pop(r)
                y_ps = ps3.tile([128, 4, E], f32, tag="y", name=f"y_ps_{r}")
                for it in range(4):
                    nc.tensor.matmul(
                        y_ps[:, it, :],
                        lhsT=out_sb[:, it * 128:(it + 1) * 128],
                        rhs=wo_sb,
                        start=True, stop=True)
                rb, ri = divmod(r, RB)
                if ri == 0:
                    y_sbs[rb] = ypool.tile([128, RB, 4, E], f32, tag="ysb",
                                           name=f"y_sb_{rb}")
                y_sb = y_sbs[rb]
                nc.vector.tensor_copy(y_sb[:, ri, 0:2, :], y_ps[:, 0:2, :])
                nc.scalar.copy(y_sb[:, ri, 2:4, :], y_ps[:, 2:4, :])
                if ri == RB - 1:
                    nc.scalar.dma_start(out=y_blk[rb], in_=y_sbs.pop(rb))

            x_tiles = {}
            for r in range(n_r + 2):
                if r < n_r:
                    rb, ri = divmod(r, RB)
                    if ri == 0:
                        x_tiles[rb] = xpool.tile([128, RB, 2, N], f16, tag="x",
                                                 name=f"x3_{rb}")
                        nc.sync.dma_start(out=x_tiles[rb], in_=xT_blk[rb])
                    stage_a(r, x_tiles[rb], ri)
                if 0 <= r - 1 < n_r:
                    stage_b(r - 1)
                if 0 <= r - 2 < n_r:
                    stage_c(r - 2)

    return nc


def _get_program():
    if "nc" not in _CACHE:
        nc = build_program()
        nc.finalize()
        _CACHE["nc"] = nc
    return _CACHE["nc"]


def make_in_maps(x, Wq, Wkv, Wo):
    """Host-side sharding: core = bi*4 + hpi."""
    scale = (64.0 ** -0.5) * (64.0 ** -0.5)
    x = np.asarray(x, np.float32)
    Wq = np.asarray(Wq, np.float32) * np.float32(scale)
    Wkv = np.asarray(Wkv, np.float32)
    Wo = np.asarray(Wo, np.float32)
    b = x.shape[0] // R
    xT = np.ascontiguousarray(
        x.reshape(b, R, N, C).transpose(0, 1, 3, 2)).astype(np.float16)
    in_maps = []
    for core in range(NCORES):
        bi, hpi = divmod(core, 4)
        cols = slice(hpi * HP, (hpi + 1) * HP)
        in_maps.append({
            "xT": xT[bi],
            "wq": np.ascontiguousarray(Wq[:, cols]).astype(np.float16),
            "wk": np.ascontiguousarray(Wkv[:, cols]).astype(np.float16),
            "wv": np.ascontiguousarray(
                Wkv[:, 512 + hpi * HP: 512 + (hpi + 1) * HP]).astype(np.float16),
            "wo": np.ascontiguousarray(Wo[cols, :]).astype(np.float16),
        })
    return in_maps


def combine_outputs(ys, bo):
    """ys: list of 8 [R, N, E] partials in core order; returns [B, n, dim]."""
    y0 = ys[0] + ys[1] + ys[2] + ys[3]
    y1 = ys[4] + ys[5] + ys[6] + ys[7]
    y = np.concatenate([y0, y1], axis=0).reshape(2 * R, N, E)
    return (y + np.asarray(bo, np.float32)).astype(np.float32)


def kernel(x, Wq, Wkv, Wo, bo, tie_attn_dim):
    assert int(tie_attn_dim) == R, f"hardcoded for tie_attn_dim={R}"
    from concourse.bass_utils import run_bass_kernel_spmd

    nc = _get_program()
    in_maps = make_in_maps(x, Wq, Wkv, Wo)
    res = run_bass_kernel_spmd(nc, in_maps, list(range(NCORES)))
    ys = [np.asarray(res.results[c]["y"], np.float32) for c in range(NCORES)]
    return combine_outputs(ys, bo)



# revision 53
# speedup vs baseline: 1.3957x; 1.3957x over previous
"""Tied-row (MSA) attention on 8 Trainium2 NeuronCores.

Reference computation (B=128, n=512, dim=256, h=8, dh=64,
r=tie_attn_dim=64, b=B//r=2):
    q = x @ Wq ; k,v = split(x @ Wkv)
    dots[b,h,i,j] = sum_{r,d} q[b,r,h,i,d] k[b,r,h,j,d] * scale
    attn = softmax_j(dots)
    out[b,r,h,i,d] = sum_j attn[b,h,i,j] v[b,r,h,j,d]
    y = out @ Wo + bo

Sharding: 8 cores = b(2) x head-pairs(4).  Each core owns one batch
element and 2 of the 8 heads and produces the partial
    y_part = out[:, :, own 2 heads, :] @ Wo[own 128 rows, :]
summed on the host (the head reduction of the output projection
commutes with the sum); bo added once.

Cost-model-driven design (timeline cost = sum over matmuls of OUTPUT
FREE SIZE x 0.4167ns; K and M are free):
  * dots contracts K=128 = (2 MSA rows x 64 d) per accumulation step
    (32 steps instead of 64), halving the dots charge.  q/k are staged
    per-head as [128=(parity,d), 32 rchunk, n] fp16.  The PSUM->SBUF
    copies cannot cross partitions, so the projection alternates
    head-swapped weight copies (wq_even / wq_odd with column halves
    swapped): every copy is then partition-identity:
      even r: ps[0:64]=h0 -> q0[0:64], ps[64:128]=h1 -> q1[64:128]
      odd  r: ps[0:64]=h1 -> q1[0:64], ps[64:128]=h0 -> q0[64:128]
  * out is computed as [i, (h d)] (lhsT=attnT chunk, rhs=v[j,(h,d)]
    slice) so the charged free axis is d=64: 32 matmuls x 64 free per
    row instead of 8 x 512.  The y projection needs out^T [(h d), i];
    that transpose is done by half-block (4 rows) xbar DMA transposes
    (16x128 tiles, 14ns each, off the engines).
  * y is written fp16 (halves the writeout on the single exclusive
    DMA server); partials are summed in f32 on the host.
  * phase-3 x blocks are prefetched during the dots wave B window so
    DMA never gates phase 3; copies are spread DVE/ACT/Pool to keep
    every engine under the PE phase time.

Per-core phases (PE charge 218.5us = model floor):
  Phase 1 (r-loop, 54.6us proj + 27.3us fused dots wave A):
    qT_r/kT_r projections -> PSUM -> split copies into q0/q1/k0/k1;
    dots chunks for i-tiles 0,1 run two rows behind.  Softmax A at
    the end of the scope (ACT exp+accum, DVE recip+mul).
  Phase 2 (27.3us): dots chunks for i-tiles 2,3; softmax B; one xbar
    transpose per head attn_h [128,4,512] -> attnT_h [j,it,jc,i];
    phase-3 x blocks 0-3 prefetched meanwhile.
  Phase 3 (109.2us, r-loop, stages A=v, B=out, C=y with C lagging 8
    rows behind the half-block transposes):
      A: v_ps[j,(4jt),(h d)] = x^T @ wv;  ACT copy -> v_sb fp16
      B: out_ps[i,4it,(h d)] += attnT_h[:,it,jc,:].T @ v_sb[:,jc,h]
         (8 interleaved PSUM groups, skip_group_check); DVE copy ->
         out_blk[i, 8r, 4it, hd] fp16; half-block DMA transpose ->
         outT_blk[hd, 8r, 4it, i]
      C: y_ps[i,4it,e] = outT.T @ wo; DVE/ACT half copies -> y_sb
         fp16; 8-row blocked DMA out on the ACT queue.

  Built with bacc.Bacc(): its compile() pass legalizes Tile's sync for
  walrus (which caps sync waits per instruction); callers must
  finalize() the program before running (see _get_program).
"""

import os
import sys

for _p in ("/opt/trn_rl_repo", "/root/.axon_site/_ro/trn_rl_repo"):
    if os.path.isdir(_p) and _p not in sys.path:
        sys.path.insert(0, _p)

import numpy as np

R = 64          # tie dim (MSA rows per batch element)
RB = 8          # rows per DMA block
RC = R // 2     # dots K-chunks (2 rows each)
N = 512         # sequence length
C = 256         # model dim
HP = 128        # head-pair width: 2 heads x 64
E = 256         # output dim
NCORES = 8
CLAG = 8        # phase-3 stage-C lag behind stage B (rows)
XH = 4          # rows per x half-block tile
NXH = R // XH   # 16 x half-blocks

_CACHE = {}


def build_program():
    import concourse.bacc as bacc
    from concourse import mybir
    from concourse.tile import TileContext
    from contextlib import ExitStack

    f32 = mybir.dt.float32
    f16 = mybir.dt.float16

    nc = bacc.Bacc()
    xT = nc.declare_dram_parameter("xT", [R, C, N], f16, isOutput=False)
    wqe = nc.declare_dram_parameter("wqe", [C, HP], f16, isOutput=False)
    wqo = nc.declare_dram_parameter("wqo", [C, HP], f16, isOutput=False)
    wke = nc.declare_dram_parameter("wke", [C, HP], f16, isOutput=False)
    wko = nc.declare_dram_parameter("wko", [C, HP], f16, isOutput=False)
    wv = nc.declare_dram_parameter("wv", [C, HP], f16, isOutput=False)
    wo = nc.declare_dram_parameter("wo", [HP, E], f16, isOutput=False)
    idm = nc.declare_dram_parameter("idm", [128, 128], f16, isOutput=False)
    y = nc.declare_dram_parameter("y", [R, N, E], f16, isOutput=True)

    # xT half-block hb viewed as [p, r_in_half, c_chunk, n]
    xT_hb = xT.rearrange("(hb r) (cc p) n -> hb p r cc n", r=XH, p=128)
    # y block rb viewed as [p, r_in_block, i_tile, e]
    y_blk = y.rearrange("(rb r) (t p) e -> rb p r t e", r=RB, p=128)

    with TileContext(nc) as tc, ExitStack() as ctx:
        singles = ctx.enter_context(tc.tile_pool(name="singles", bufs=1))
        sm = ctx.enter_context(tc.tile_pool(name="sm", bufs=4))
        attntp = ctx.enter_context(tc.tile_pool(name="attntp", bufs=2))
        xpool = ctx.enter_context(tc.tile_pool(name="xpool", bufs=6))
        # one PSUM pool for the whole kernel: pool-scope closes would force
        # phase-3 bank allocations to wait on ALL phase-1/2 readers; with
        # shared tags phase 3 lands on the banks softmax A freed early
        ps = ctx.enter_context(tc.tile_pool(name="ps", space="PSUM", bufs=2))

        # weights first, one DMA each, split across the two HWDGE queues
        # (SP + ACT) so they land in ~2us; then x block 0 in 2-row slices
        # so the first projection starts ~2us after that.
        wq_sb = [singles.tile([128, 2, HP], f16, name=f"wq{p}") for p in range(2)]
        wk_sb = [singles.tile([128, 2, HP], f16, name=f"wk{p}") for p in range(2)]
        wv_sb = singles.tile([128, 2, HP], f16)
        wo_sb = singles.tile([128, E], f16)
        w2 = "(cc p) h -> p cc h"
        nc.sync.dma_start(out=wq_sb[0], in_=wqe.rearrange(w2, p=128))
        nc.scalar.dma_start(out=wq_sb[1], in_=wqo.rearrange(w2, p=128))
        nc.sync.dma_start(out=wk_sb[0], in_=wke.rearrange(w2, p=128))
        nc.scalar.dma_start(out=wk_sb[1], in_=wko.rearrange(w2, p=128))

        # x streams in 4-row half-blocks; the first half in 2-row slices on
        # SP while ACT brings half 1 (then the phase-3-only wv/wo)
        x_sb0 = xpool.tile([128, XH, 2, N], f16, tag="x", name="x1_0")
        nc.sync.dma_start(out=x_sb0[:, 0:2], in_=xT_hb[0, :, 0:2])
        nc.scalar.dma_start(out=wv_sb, in_=wv.rearrange(w2, p=128))
        nc.scalar.dma_start(out=wo_sb, in_=wo[:, :])
        nc.sync.dma_start(out=x_sb0[:, 2:4], in_=xT_hb[0, :, 2:4])
        x_sb1 = xpool.tile([128, XH, 2, N], f16, tag="x", name="x1_1")
        nc.scalar.dma_start(out=x_sb1[:, 0:2], in_=xT_hb[1, :, 0:2])
        nc.scalar.dma_start(out=x_sb1[:, 2:4], in_=xT_hb[1, :, 2:4])

        # attnT survives into phase 3: kernel-scoped pool
        # layout [j_in_chunk, it, jc, i_in_tile]
        attnT = [attntp.tile([128, 4, 4, 128], f16, tag="attnT",
                             name=f"attnT_{h}") for h in range(2)]

        def softmax(dots_hit, attn_dst):
            """dots PSUM tile -> normalized f16 attn slice.

            No max-subtraction: dots = q k^T with the 1/(sqrt(dh) sqrt(r))
            scale folded into Wq, so entries are ~N(0,1) and exp cannot
            overflow fp32/fp16."""
            ssum = sm.tile([128, 1], f32, tag="ssum", bufs=8)
            rinv = sm.tile([128, 1], f32, tag="rinv", bufs=8)
            nc.scalar.activation(
                out=attn_dst, in_=dots_hit,
                func=mybir.ActivationFunctionType.Exp,
                accum_out=ssum)
            nc.vector.reciprocal(rinv, ssum)
            nc.vector.tensor_scalar_mul(attn_dst, attn_dst, rinv)

        if True:
            # attn_h [i_in_tile, it, j]; one tile per head so the transpose
            # to attnT_h is a single xbar DMA per head (kernel-scoped pool:
            # a pool close before phase 3 would barrier DVE on the late
            # attn transposes)
            attn = [attntp.tile([128, 4, N], f16, name=f"attn_{h}")
                    for h in range(2)]

            # resident per-head K-packed fp16 q/k, phases 1-2 only
            with tc.tile_pool(name="resid", bufs=1) as resid:
                qh = [resid.tile([128, RC, N], f16, name=f"q{h}")
                      for h in range(2)]
                kh = [resid.tile([128, RC, N], f16, name=f"k{h}")
                      for h in range(2)]

                def dots_chunk(dots_tiles, c, its):
                    for h in range(2):
                        for it in its:
                            nc.tensor.matmul(
                                dots_tiles[h][it % 2],
                                lhsT=qh[h][:, c, it * 128:(it + 1) * 128],
                                rhs=kh[h][:, c, :],
                                start=(c == 0), stop=(c == RC - 1))

                # ---- Phase 1 + dots wave A (i-tiles 0,1) fused;
                # ---- wave B (i-tiles 2,3) reuses the q/k PSUM banks and
                # ---- runs before softmax A is emitted so the ACT exps of
                # ---- wave A overlap wave B's accumulation.
                x3 = {}
                if True:
                    dotsA = [[ps.tile([128, N], f32, tag="dots", bufs=4,
                                       name=f"dotsA_{h}_{it}")
                              for it in range(2)] for h in range(2)]
                    x_sb = x_sb0
                    for r in range(R):
                        hb, ri = divmod(r, XH)
                        if ri == 0 and hb == 1:
                            x_sb = x_sb1
                        elif ri == 0 and hb > 1:
                            x_sb = xpool.tile([128, XH, 2, N], f16, tag="x",
                                              name=f"x1_{hb}")
                            nc.sync.dma_start(out=x_sb, in_=xT_hb[hb])
                        # prefetch phase-3 x halves 0-3 into the slots the
                        # phase-1 stream has finished with
                        if r >= 48 and ri == 0:
                            hb3 = (r - 48) // 4
                            x3[hb3] = xpool.tile([128, XH, 2, N], f16, tag="x",
                                                 name=f"x3_{hb3}")
                            nc.sync.dma_start(out=x3[hb3], in_=xT_hb[hb3])
                        par = r % 2
                        rc = r // 2
                        q_ps = ps.tile([128, N], f32, tag="q")
                        k_ps = ps.tile([128, N], f32, tag="k")
                        for cc in range(2):
                            nc.tensor.matmul(q_ps, lhsT=wq_sb[par][:, cc, :],
                                             rhs=x_sb[:, ri, cc, :],
                                             start=(cc == 0), stop=(cc == 1))
                        for cc in range(2):
                            nc.tensor.matmul(k_ps, lhsT=wk_sb[par][:, cc, :],
                                             rhs=x_sb[:, ri, cc, :],
                                             start=(cc == 0), stop=(cc == 1))
                        # partition-identity split copies (see module doc):
                        # lo half -> head `par`, hi half -> head `1-par`
                        lo, hi = (0, 1) if par == 0 else (1, 0)
                        nc.vector.tensor_copy(
                            qh[lo][0:64, rc, :], q_ps[0:64, :])
                        nc.vector.tensor_copy(
                            qh[hi][64:128, rc, :], q_ps[64:128, :])
                        nc.scalar.copy(kh[lo][0:64, rc, :], k_ps[0:64, :])
                        nc.scalar.copy(kh[hi][64:128, rc, :], k_ps[64:128, :])
                        # wave A dots, two rows behind the copies
                        if par == 1 and r >= 3:
                            dots_chunk(dotsA, (r - 3) // 2, (0, 1))
                    dots_chunk(dotsA, RC - 1, (0, 1))

                    # wave B accumulators take over the q/k bank slots
                    dotsB = [[ps.tile([128, N], f32, tag=t, name=f"dotsB_{h}_{it}")
                              for it, t in ((2, "q"), (3, "k"))] for h in range(2)]
                    dotsB = [{2: dotsB[h][0], 3: dotsB[h][1]} for h in range(2)]
                    for c in range(RC):
                        for h in range(2):
                            for it in (2, 3):
                                nc.tensor.matmul(
                                    dotsB[h][it],
                                    lhsT=qh[h][:, c, it * 128:(it + 1) * 128],
                                    rhs=kh[h][:, c, :],
                                    start=(c == 0), stop=(c == RC - 1))
                        if c in (4, 20):  # phase-3 x halves 4,5 prefetched
                            hb3 = {4: 4, 20: 5}[c]
                            x3[hb3] = xpool.tile([128, XH, 2, N], f16, tag="x",
                                                 name=f"x3_{hb3}")
                            nc.sync.dma_start(out=x3[hb3], in_=xT_hb[hb3])
                    # softmax A overlaps wave B on ACT/DVE, and its half of
                    # attnT transposes during wave B too; only the B half
                    # (i-tiles 2,3) is on the wave-B critical path
                    for h in range(2):
                        for it in range(2):
                            softmax(dotsA[h][it], attn[h][:, it, :])
                    for h in range(2):
                        nc.sync.dma_start_transpose(
                            out=attnT[h][:, 0:2], in_=attn[h][:, 0:2, :])
                    for h in range(2):
                        for it in range(2, 4):
                            softmax(dotsB[h][it], attn[h][:, it, :])
                        nc.sync.dma_start_transpose(
                            out=attnT[h][:, 2:4], in_=attn[h][:, 2:4, :])

        # ------- Phase 3: v, out, outT (PE transpose), y ------------------
        # The out^T transposes run on the PE itself (is_transpose matmuls,
        # 53ns per 128x128 fp16 tile): no DMA-server or cross-queue latency
        # in the B -> y critical chain, so stage lags are short.  PSUM: v(2)
        # + out(2) + T fp16(2 half-banks) + 2 y half-tiles = 8 banks.
        with tc.tile_pool(name="vpool", bufs=12) as vpool, \
             tc.tile_pool(name="outp", bufs=5) as outp, \
             tc.tile_pool(name="outtp", bufs=5) as outtp, \
             tc.tile_pool(name="ypool", bufs=2) as ypool:
            ident = singles.tile([128, 128], f16, name="ident")
            nc.scalar.dma_start(out=ident, in_=idm[:, :])
            v_sbs = {}
            out_sbs = {}
            outT_sbs = {}
            y_sbs = {}

            def stage_a(r):
                hb, ri = divmod(r, XH)
                # issue-ahead of 4 halves: the reused buffer slot's readers
                # retired ~9 rows ago, so this DMA never parks on the SP queue
                if ri == 0 and 6 <= hb + 4 < NXH:
                    x3[hb + 4] = xpool.tile([128, XH, 2, N], f16, tag="x",
                                            name=f"x3_{hb + 4}")
                    nc.sync.dma_start(out=x3[hb + 4], in_=xT_hb[hb + 4])
                v_ps = ps.tile([128, 4, 128], f32, tag="dots", bufs=4,
                                name=f"v_ps_{r}")
                for jt in range(4):
                    for cc in range(2):
                        nc.tensor.matmul(
                            v_ps[:, jt, :],
                            lhsT=x3[hb][:, ri, cc, jt * 128:(jt + 1) * 128],
                            rhs=wv_sb[:, cc, :],
                            start=(cc == 0), stop=(cc == 1))
                v_sb = vpool.tile([128, 4, 128], f16, tag="vsb", name=f"v_sb_{r}")
                # entry rows copy on DVE (idle until B(0)) so the A-lead can
                # run ahead while ACT drains softmax B
                if r < 9:
                    nc.vector.tensor_copy(v_sb, v_ps)
                else:
                    nc.scalar.copy(v_sb, v_ps)
                v_sbs[r] = v_sb

            def stage_b(r):
                v_sb = v_sbs.pop(r)
                out_ps = ps.tile([128, 4, 128], f32, tag="dots", bufs=4,
                                 name=f"out_ps_{r}")
                for it in range(4):
                    for h in range(2):
                        hs = slice(h * 64, (h + 1) * 64)
                        for jc in range(4):
                            nc.tensor.matmul(
                                out_ps[:, it, hs],
                                lhsT=attnT[h][:, it, jc, :],
                                rhs=v_sb[:, jc, hs],
                                start=(jc == 0), stop=(jc == 3),
                                skip_group_check=True)
                out_sb = outp.tile([128, 4, 128], f16, tag="ob",
                                   name=f"out_sb_{r}")
                nc.vector.tensor_copy(out_sb, out_ps)
                out_sbs[r] = out_sb

            def stage_t(r):
                out_sb = out_sbs.pop(r)
                # padded to a full 2KB PSUM bank so the shared "q" tag
                # keeps a single tile size
                t_ps = ps.tile([128, 8, 128], f16, tag="q", name=f"t_ps_{r}")
                for it in range(4):
                    nc.tensor.transpose(t_ps[:, it, :], out_sb[:, it, :], ident)
                outT = outtp.tile([128, 4, 128], f16, tag="ot",
                                  name=f"outT_{r}")
                nc.scalar.copy(outT, t_ps[:, 0:4, :])
                outT_sbs[r] = outT

            def stage_c(r):
                rb, ri = divmod(r, RB)
                outT = outT_sbs.pop(r)
                if ri == 0:
                    y_sbs[rb] = ypool.tile([128, RB, 4, E], f16, tag="ysb",
                                           name=f"y_sb_{rb}")
                y_sb = y_sbs[rb]
                # two 1-bank PSUM halves; copies split ACT/Pool (off DVE so
                # out/T copies flow without queueing delay)
                y_psa = ps.tile([128, 2, E], f32, tag="q", name=f"y_psa_{r}")
                for it in range(2):
                    nc.tensor.matmul(y_psa[:, it, :], lhsT=outT[:, it, :],
                                     rhs=wo_sb, start=True, stop=True)
                nc.vector.tensor_copy(y_sb[:, ri, 0:2, :], y_psa)
                y_psb = ps.tile([128, 2, E], f32, tag="k", name=f"y_psb_{r}")
                for it in range(2, 4):
                    nc.tensor.matmul(y_psb[:, it - 2, :], lhsT=outT[:, it, :],
                                     rhs=wo_sb, start=True, stop=True)
                nc.scalar.copy(y_sb[:, ri, 2:4, :], y_psb)
                # half-block writeout on the SWDGE (Pool) queue; the last
                # block drains in 2-row pieces, the final ones on the idle
                # SP/HWDGE queue (~1.2us less SWDGE generation each)
                if rb == RB - 1:
                    pieces = {3: 0, 5: 4, 7: 6}  # ri -> start row
                else:
                    pieces = {3: 0, 7: 4}
                if ri in pieces:
                    lo = pieces[ri]
                    if rb == RB - 1 and ri >= 5:
                        nc.sync.dma_start(out=y_blk[rb, :, lo:ri + 1],
                                          in_=y_sb[:, lo:ri + 1])
                    else:
                        nc.gpsimd.dma_start(out=y_blk[rb, :, lo:ri + 1],
                                            in_=y_sb[:, lo:ri + 1])
                    if ri == RB - 1:
                        y_sbs.pop(rb)

            # stage A leads by 9 rows: the v projections (independent of
            # attnT) keep the PE busy through the softmax-B -> attnT
            # transpose latency at phase-3 entry
            for r in range(R + 12):
                if r < R:
                    stage_a(r)
                if 0 <= r - 9 < R:
                    stage_b(r - 9)
                if 0 <= r - 11 < R:
                    stage_t(r - 11)
                if 0 <= r - 12 < R:
                    stage_c(r - 12)

    return nc


def _get_program():
    if "nc" not in _CACHE:
        nc = build_program()
        nc.finalize()
        _CACHE["nc"] = nc
    return _CACHE["nc"]


def make_in_maps(x, Wq, Wkv, Wo):
    """Host-side sharding: core = bi*4 + hpi."""
    scale = (64.0 ** -0.5) * (64.0 ** -0.5)
    x = np.asarray(x, np.float32)
    Wq = np.asarray(Wq, np.float32) * np.float32(scale)
    Wkv = np.asarray(Wkv, np.float32)
    Wo = np.asarray(Wo, np.float32)
    b = x.shape[0] // R
    xT = np.ascontiguousarray(
        x.reshape(b, R, N, C).transpose(0, 1, 3, 2)).astype(np.float16)

    def swap_heads(w):  # [C, 128] -> column halves swapped
        return np.ascontiguousarray(
            np.concatenate([w[:, 64:], w[:, :64]], axis=1))

    in_maps = []
    for core in range(NCORES):
        bi, hpi = divmod(core, 4)
        cols = slice(hpi * HP, (hpi + 1) * HP)
        wq_c = np.ascontiguousarray(Wq[:, cols]).astype(np.float16)
        wk_c = np.ascontiguousarray(Wkv[:, cols]).astype(np.float16)
        in_maps.append({
            "xT": xT[bi],
            "idm": np.eye(128, dtype=np.float16),
            "wqe": wq_c,
            "wqo": swap_heads(wq_c),
            "wke": wk_c,
            "wko": swap_heads(wk_c),
            "wv": np.ascontiguousarray(
                Wkv[:, 512 + hpi * HP: 512 + (hpi + 1) * HP]).astype(np.float16),
            "wo": np.ascontiguousarray(Wo[cols, :]).astype(np.float16),
        })
    return in_maps


def combine_outputs(ys, bo):
    """ys: list of 8 [R, N, E] partials in core order; returns [B, n, dim]."""
    ys = [np.asarray(t, np.float32) for t in ys]
    y0 = ys[0] + ys[1] + ys[2] + ys[3]
    y1 = ys[4] + ys[5] + ys[6] + ys[7]
    y = np.concatenate([y0, y1], axis=0).reshape(2 * R, N, E)
    return (y + np.asarray(bo, np.float32)).astype(np.float32)


def kernel(x, Wq, Wkv, Wo, bo, tie_attn_dim):
    assert int(tie_attn_dim) == R, f"hardcoded for tie_attn_dim={R}"
    from concourse.bass_utils import run_bass_kernel_spmd

    nc = _get_program()
    in_maps = make_in_maps(x, Wq, Wkv, Wo)
    res = run_bass_kernel_spmd(nc, in_maps, list(range(NCORES)))
    ys = [res.results[c]["y"] for c in range(NCORES)]
    return combine_outputs(ys, bo)


# revision 55
# speedup vs baseline: 1.4539x; 1.0416x over previous
"""Tied-row (MSA) attention on 8 Trainium2 NeuronCores.

Reference computation (B=128, n=512, dim=256, h=8, dh=64,
r=tie_attn_dim=64, b=B//r=2):
    q = x @ Wq ; k,v = split(x @ Wkv)
    dots[b,h,i,j] = sum_{r,d} q[b,r,h,i,d] k[b,r,h,j,d] * scale
    attn = softmax_j(dots)
    out[b,r,h,i,d] = sum_j attn[b,h,i,j] v[b,r,h,j,d]
    y = out @ Wo + bo

Sharding: 8 cores = b(2) x head-pairs(4).  Each core owns one batch
element and 2 of the 8 heads and produces the partial
    y_part = out[:, :, own 2 heads, :] @ Wo[own 128 rows, :]
summed on the host (the head reduction of the output projection
commutes with the sum); bo added once.

Cost-model-driven design (timeline cost = sum over matmuls of OUTPUT
FREE SIZE x 0.4167ns; K and M are free):
  * dots contracts K=128 = (2 MSA rows x 64 d) per accumulation step
    (32 steps instead of 64), halving the dots charge.  q/k are staged
    per-head as [128=(parity,d), 32 rchunk, n] fp16.  The PSUM->SBUF
    copies cannot cross partitions, so the projection alternates
    head-swapped weight copies (wq_even / wq_odd with column halves
    swapped): every copy is then partition-identity:
      even r: ps[0:64]=h0 -> q0[0:64], ps[64:128]=h1 -> q1[64:128]
      odd  r: ps[0:64]=h1 -> q1[0:64], ps[64:128]=h0 -> q0[64:128]
  * out is computed as [i, (h d)] (lhsT=attnT chunk, rhs=v[j,(h,d)]
    slice) so the charged free axis is d=64: 32 matmuls x 64 free per
    row instead of 8 x 512.  The y projection needs out^T [(h d), i];
    that transpose is done by half-block (4 rows) xbar DMA transposes
    (16x128 tiles, 14ns each, off the engines).
  * y is written fp16 (halves the writeout on the single exclusive
    DMA server); partials are summed in f32 on the host.
  * phase-3 x blocks are prefetched during the dots wave B window so
    DMA never gates phase 3; copies are spread DVE/ACT/Pool to keep
    every engine under the PE phase time.

Per-core phases (PE charge 218.5us = model floor):
  Phase 1 (r-loop, 54.6us proj + 27.3us fused dots wave A):
    qT_r/kT_r projections -> PSUM -> split copies into q0/q1/k0/k1;
    dots chunks for i-tiles 0,1 run two rows behind.  Softmax A at
    the end of the scope (ACT exp+accum, DVE recip+mul).
  Phase 2 (27.3us): dots chunks for i-tiles 2,3; softmax B; one xbar
    transpose per head attn_h [128,4,512] -> attnT_h [j,it,jc,i];
    phase-3 x blocks 0-3 prefetched meanwhile.
  Phase 3 (109.2us, r-loop, stages A=v, B=out, C=y with C lagging 8
    rows behind the half-block transposes):
      A: v_ps[j,(4jt),(h d)] = x^T @ wv;  ACT copy -> v_sb fp16
      B: out_ps[i,4it,(h d)] += attnT_h[:,it,jc,:].T @ v_sb[:,jc,h]
         (8 interleaved PSUM groups, skip_group_check); DVE copy ->
         out_blk[i, 8r, 4it, hd] fp16; half-block DMA transpose ->
         outT_blk[hd, 8r, 4it, i]
      C: y_ps[i,4it,e] = outT.T @ wo; DVE/ACT half copies -> y_sb
         fp16; 8-row blocked DMA out on the ACT queue.

  Built with bacc.Bacc(): its compile() pass legalizes Tile's sync for
  walrus (which caps sync waits per instruction); callers must
  finalize() the program before running (see _get_program).
"""

import os
import sys

for _p in ("/opt/trn_rl_repo", "/root/.axon_site/_ro/trn_rl_repo"):
    if os.path.isdir(_p) and _p not in sys.path:
        sys.path.insert(0, _p)

import numpy as np

R = 64          # tie dim (MSA rows per batch element)
RB = 8          # rows per DMA block
RC = R // 2     # dots K-chunks (2 rows each)
N = 512         # sequence length
C = 256         # model dim
HP = 128        # head-pair width: 2 heads x 64
E = 256         # output dim
NCORES = 8
CLAG = 8        # phase-3 stage-C lag behind stage B (rows)
XH = 4          # rows per x half-block tile
NXH = R // XH   # 16 x half-blocks

_CACHE = {}


def build_program():
    import concourse.bacc as bacc
    from concourse import mybir
    from concourse.tile import TileContext
    from contextlib import ExitStack

    f32 = mybir.dt.float32
    f16 = mybir.dt.float16

    nc = bacc.Bacc()
    xT = nc.declare_dram_parameter("xT", [R, C, N], f16, isOutput=False)
    wqe = nc.declare_dram_parameter("wqe", [C, HP], f16, isOutput=False)
    wqo = nc.declare_dram_parameter("wqo", [C, HP], f16, isOutput=False)
    wke = nc.declare_dram_parameter("wke", [C, HP], f16, isOutput=False)
    wko = nc.declare_dram_parameter("wko", [C, HP], f16, isOutput=False)
    wv = nc.declare_dram_parameter("wv", [C, HP], f16, isOutput=False)
    wo = nc.declare_dram_parameter("wo", [HP, E], f16, isOutput=False)
    idm = nc.declare_dram_parameter("idm", [128, 128], f16, isOutput=False)
    y = nc.declare_dram_parameter("y", [R, N, E], f16, isOutput=True)

    # xT half-block hb viewed as [p, r_in_half, c_chunk, n]
    xT_hb = xT.rearrange("(hb r) (cc p) n -> hb p r cc n", r=XH, p=128)
    # y block rb viewed as [p, r_in_block, i_tile, e]
    y_blk = y.rearrange("(rb r) (t p) e -> rb p r t e", r=RB, p=128)

    with TileContext(nc) as tc, ExitStack() as ctx:
        singles = ctx.enter_context(tc.tile_pool(name="singles", bufs=1))
        sm = ctx.enter_context(tc.tile_pool(name="sm", bufs=4))
        attntp = ctx.enter_context(tc.tile_pool(name="attntp", bufs=2))
        xpool = ctx.enter_context(tc.tile_pool(name="xpool", bufs=6))
        stp = ctx.enter_context(tc.tile_pool(name="stp", bufs=2))
        # one PSUM pool for the whole kernel: pool-scope closes would force
        # phase-3 bank allocations to wait on ALL phase-1/2 readers; with
        # shared tags phase 3 lands on the banks softmax A freed early
        ps = ctx.enter_context(tc.tile_pool(name="ps", space="PSUM", bufs=2))

        # weights first, one DMA each, split across the two HWDGE queues
        # (SP + ACT) so they land in ~2us; then x block 0 in 2-row slices
        # so the first projection starts ~2us after that.
        wq_sb = [singles.tile([128, 2, HP], f16, name=f"wq{p}") for p in range(2)]
        wk_sb = [singles.tile([128, 2, HP], f16, name=f"wk{p}") for p in range(2)]
        wv_sb = singles.tile([128, 2, HP], f16)
        wo_sb = singles.tile([128, E], f16)
        # x rows 0-3 race in first on both queues, then weights, x half 1,
        # and the phase-3-only wv/wo
        w2 = "(cc p) h -> p cc h"
        x_sb0 = xpool.tile([128, XH, 2, N], f16, tag="x", name="x1_0")
        nc.sync.dma_start(out=x_sb0[:, 0:2], in_=xT_hb[0, :, 0:2])
        nc.scalar.dma_start(out=wq_sb[1], in_=wqo.rearrange(w2, p=128))
        nc.sync.dma_start(out=wq_sb[0], in_=wqe.rearrange(w2, p=128))
        nc.scalar.dma_start(out=wk_sb[1], in_=wko.rearrange(w2, p=128))
        nc.sync.dma_start(out=wk_sb[0], in_=wke.rearrange(w2, p=128))
        nc.scalar.dma_start(out=x_sb0[:, 2:4], in_=xT_hb[0, :, 2:4])
        x_sb1 = xpool.tile([128, XH, 2, N], f16, tag="x", name="x1_1")
        nc.sync.dma_start(out=x_sb1[:, 0:2], in_=xT_hb[1, :, 0:2])
        nc.scalar.dma_start(out=x_sb1[:, 2:4], in_=xT_hb[1, :, 2:4])
        nc.scalar.dma_start(out=wv_sb, in_=wv.rearrange(w2, p=128))
        nc.scalar.dma_start(out=wo_sb, in_=wo[:, :])

        # attnT survives into phase 3: kernel-scoped pool
        # layout [j_in_chunk, it, jc, i_in_tile]
        attnT = [attntp.tile([128, 4, 4, 128], f16, tag="attnT",
                             name=f"attnT_{h}") for h in range(2)]

        def softmax(dots_hit, attn_dst):
            """dots PSUM tile -> normalized f16 attn slice.

            No max-subtraction: dots = q k^T with the 1/(sqrt(dh) sqrt(r))
            scale folded into Wq, so entries are ~N(0,1) and exp cannot
            overflow fp32/fp16."""
            ssum = sm.tile([128, 1], f32, tag="ssum", bufs=8)
            rinv = sm.tile([128, 1], f32, tag="rinv", bufs=8)
            nc.scalar.activation(
                out=attn_dst, in_=dots_hit,
                func=mybir.ActivationFunctionType.Exp,
                accum_out=ssum)
            nc.vector.reciprocal(rinv, ssum)
            nc.vector.tensor_scalar_mul(attn_dst, attn_dst, rinv)

        if True:
            # attn_h [i_in_tile, it, j]; one tile per head so the transpose
            # to attnT_h is a single xbar DMA per head (kernel-scoped pool:
            # a pool close before phase 3 would barrier DVE on the late
            # attn transposes)
            attn = [attntp.tile([128, 4, N], f16, name=f"attn_{h}")
                    for h in range(2)]

            # resident per-head K-packed fp16 q/k, phases 1-2 only
            with tc.tile_pool(name="resid", bufs=1) as resid:
                qh = [resid.tile([128, RC, N], f16, name=f"q{h}")
                      for h in range(2)]
                kh = [resid.tile([128, RC, N], f16, name=f"k{h}")
                      for h in range(2)]

                def dots_chunk(dots_tiles, c, its):
                    for h in range(2):
                        for it in its:
                            nc.tensor.matmul(
                                dots_tiles[h][it % 2],
                                lhsT=qh[h][:, c, it * 128:(it + 1) * 128],
                                rhs=kh[h][:, c, :],
                                start=(c == 0), stop=(c == RC - 1))

                # ---- Phase 1 + dots wave A (i-tiles 0,1) fused;
                # ---- wave B (i-tiles 2,3) reuses the q/k PSUM banks and
                # ---- runs before softmax A is emitted so the ACT exps of
                # ---- wave A overlap wave B's accumulation.
                x3 = {}
                if True:
                    dotsA = [[ps.tile([128, N], f32, tag="dots", bufs=4,
                                       name=f"dotsA_{h}_{it}")
                              for it in range(2)] for h in range(2)]
                    x_sb = x_sb0
                    for r in range(R):
                        hb, ri = divmod(r, XH)
                        if ri == 0 and hb == 1:
                            x_sb = x_sb1
                        elif ri == 0 and hb > 1:
                            x_sb = xpool.tile([128, XH, 2, N], f16, tag="x",
                                              name=f"x1_{hb}")
                            nc.sync.dma_start(out=x_sb, in_=xT_hb[hb])
                        # prefetch phase-3 x halves 0-3 into the slots the
                        # phase-1 stream has finished with
                        if r >= 48 and ri == 0:
                            hb3 = (r - 48) // 4
                            x3[hb3] = xpool.tile([128, XH, 2, N], f16, tag="x",
                                                 name=f"x3_{hb3}")
                            nc.sync.dma_start(out=x3[hb3], in_=xT_hb[hb3])
                        par = r % 2
                        rc = r // 2
                        q_ps = ps.tile([128, N], f32, tag="q")
                        k_ps = ps.tile([128, N], f32, tag="k")
                        for cc in range(2):
                            nc.tensor.matmul(q_ps, lhsT=wq_sb[par][:, cc, :],
                                             rhs=x_sb[:, ri, cc, :],
                                             start=(cc == 0), stop=(cc == 1))
                        for cc in range(2):
                            nc.tensor.matmul(k_ps, lhsT=wk_sb[par][:, cc, :],
                                             rhs=x_sb[:, ri, cc, :],
                                             start=(cc == 0), stop=(cc == 1))
                        # partition-identity split copies (see module doc):
                        # lo half -> head `par`, hi half -> head `1-par`.
                        # q goes PSUM -> f16 staging (DVE 1x) then two cheap
                        # f16 SBUF splits (DVE 2x mode + Pool), keeping every
                        # engine under the phase-1 PE time; k copies direct
                        # on ACT.  (Pool cannot read PSUM on walrus.)
                        lo, hi = (0, 1) if par == 0 else (1, 0)
                        q_st = stp.tile([128, N], f16, tag="qst", name=f"qst_{r}")
                        nc.vector.tensor_copy(q_st, q_ps)
                        nc.vector.tensor_copy(
                            qh[lo][0:64, rc, :], q_st[0:64, :])
                        nc.gpsimd.tensor_copy(
                            qh[hi][64:128, rc, :], q_st[64:128, :])
                        nc.scalar.copy(kh[lo][0:64, rc, :], k_ps[0:64, :])
                        nc.scalar.copy(kh[hi][64:128, rc, :], k_ps[64:128, :])
                        # wave A dots, two rows behind the copies
                        if par == 1 and r >= 3:
                            dots_chunk(dotsA, (r - 3) // 2, (0, 1))
                    dots_chunk(dotsA, RC - 1, (0, 1))

                    # wave B accumulators take over the q/k bank slots
                    dotsB = [[ps.tile([128, N], f32, tag=t, name=f"dotsB_{h}_{it}")
                              for it, t in ((2, "q"), (3, "k"))] for h in range(2)]
                    dotsB = [{2: dotsB[h][0], 3: dotsB[h][1]} for h in range(2)]
                    for c in range(RC):
                        for h in range(2):
                            for it in (2, 3):
                                nc.tensor.matmul(
                                    dotsB[h][it],
                                    lhsT=qh[h][:, c, it * 128:(it + 1) * 128],
                                    rhs=kh[h][:, c, :],
                                    start=(c == 0), stop=(c == RC - 1))
                        if c in (4, 20):  # phase-3 x halves 4,5 prefetched
                            hb3 = {4: 4, 20: 5}[c]
                            x3[hb3] = xpool.tile([128, XH, 2, N], f16, tag="x",
                                                 name=f"x3_{hb3}")
                            nc.sync.dma_start(out=x3[hb3], in_=xT_hb[hb3])
                    # softmax A overlaps wave B on ACT/DVE, and its half of
                    # attnT transposes during wave B too; only the B half
                    # (i-tiles 2,3) is on the wave-B critical path
                    for h in range(2):
                        for it in range(2):
                            softmax(dotsA[h][it], attn[h][:, it, :])
                    for h in range(2):
                        nc.sync.dma_start_transpose(
                            out=attnT[h][:, 0:2], in_=attn[h][:, 0:2, :])
                    for h in range(2):
                        for it in range(2, 4):
                            softmax(dotsB[h][it], attn[h][:, it, :])
                        nc.sync.dma_start_transpose(
                            out=attnT[h][:, 2:4], in_=attn[h][:, 2:4, :])

        # ------- Phase 3: v, out, outT (PE transpose), y ------------------
        # The out^T transposes run on the PE itself (is_transpose matmuls,
        # 53ns per 128x128 fp16 tile): no DMA-server or cross-queue latency
        # in the B -> y critical chain, so stage lags are short.  PSUM: v(2)
        # + out(2) + T fp16(2 half-banks) + 2 y half-tiles = 8 banks.
        with tc.tile_pool(name="vpool", bufs=12) as vpool, \
             tc.tile_pool(name="outp", bufs=5) as outp, \
             tc.tile_pool(name="outtp", bufs=5) as outtp, \
             tc.tile_pool(name="ypool", bufs=2) as ypool:
            ident = singles.tile([128, 128], f16, name="ident")
            nc.scalar.dma_start(out=ident, in_=idm[:, :])
            v_sbs = {}
            out_sbs = {}
            outT_sbs = {}
            y_sbs = {}

            def stage_a(r):
                hb, ri = divmod(r, XH)
                # issue-ahead of 4 halves: the reused buffer slot's readers
                # retired ~9 rows ago, so this DMA never parks on the SP queue
                if ri == 0 and 6 <= hb + 4 < NXH:
                    x3[hb + 4] = xpool.tile([128, XH, 2, N], f16, tag="x",
                                            name=f"x3_{hb + 4}")
                    nc.sync.dma_start(out=x3[hb + 4], in_=xT_hb[hb + 4])
                v_ps = ps.tile([128, 4, 128], f32, tag="dots", bufs=4,
                                name=f"v_ps_{r}")
                for jt in range(4):
                    for cc in range(2):
                        nc.tensor.matmul(
                            v_ps[:, jt, :],
                            lhsT=x3[hb][:, ri, cc, jt * 128:(jt + 1) * 128],
                            rhs=wv_sb[:, cc, :],
                            start=(cc == 0), stop=(cc == 1))
                v_sb = vpool.tile([128, 4, 128], f16, tag="vsb", name=f"v_sb_{r}")
                # entry rows copy on DVE (idle until B(0)) so the A-lead can
                # run ahead while ACT drains softmax B
                if r < 9:
                    nc.vector.tensor_copy(v_sb, v_ps)
                else:
                    nc.scalar.copy(v_sb, v_ps)
                v_sbs[r] = v_sb

            out_pss = {}

            def stage_b(r, its=(0, 1, 2, 3), done=True):
                if r in out_pss:
                    out_ps = out_pss[r]
                else:
                    out_ps = out_pss[r] = ps.tile(
                        [128, 4, 128], f32, tag="dots", bufs=4,
                        name=f"out_ps_{r}")
                for it in its:
                    for h in range(2):
                        hs = slice(h * 64, (h + 1) * 64)
                        for jc in range(4):
                            nc.tensor.matmul(
                                out_ps[:, it, hs],
                                lhsT=attnT[h][:, it, jc, :],
                                rhs=v_sbs[r][:, jc, hs],
                                start=(jc == 0), stop=(jc == 3),
                                skip_group_check=True)
                if done:
                    v_sbs.pop(r)
                    out_pss.pop(r)
                    out_sb = outp.tile([128, 4, 128], f16, tag="ob",
                                       name=f"out_sb_{r}")
                    nc.vector.tensor_copy(out_sb, out_ps)
                    out_sbs[r] = out_sb

            def stage_t(r):
                out_sb = out_sbs.pop(r)
                # padded to a full 2KB PSUM bank so the shared "q" tag
                # keeps a single tile size
                t_ps = ps.tile([128, 8, 128], f16, tag="q", name=f"t_ps_{r}")
                for it in range(4):
                    nc.tensor.transpose(t_ps[:, it, :], out_sb[:, it, :], ident)
                outT = outtp.tile([128, 4, 128], f16, tag="ot",
                                  name=f"outT_{r}")
                nc.scalar.copy(outT, t_ps[:, 0:4, :])
                outT_sbs[r] = outT

            def stage_c(r):
                rb, ri = divmod(r, RB)
                outT = outT_sbs.pop(r)
                if ri == 0:
                    y_sbs[rb] = ypool.tile([128, RB, 4, E], f16, tag="ysb",
                                           name=f"y_sb_{rb}")
                y_sb = y_sbs[rb]
                # two 1-bank PSUM halves; copies split ACT/Pool (off DVE so
                # out/T copies flow without queueing delay)
                y_psa = ps.tile([128, 2, E], f32, tag="k", name=f"y_psa_{r}")
                for it in range(2):
                    nc.tensor.matmul(y_psa[:, it, :], lhsT=outT[:, it, :],
                                     rhs=wo_sb, start=True, stop=True)
                nc.vector.tensor_copy(y_sb[:, ri, 0:2, :], y_psa)
                y_psb = ps.tile([128, 2, E], f32, tag="k", name=f"y_psb_{r}")
                for it in range(2, 4):
                    nc.tensor.matmul(y_psb[:, it - 2, :], lhsT=outT[:, it, :],
                                     rhs=wo_sb, start=True, stop=True)
                nc.scalar.copy(y_sb[:, ri, 2:4, :], y_psb)
                # half-block writeout on the SWDGE (Pool) queue; the last
                # block drains in 2-row pieces, the final ones on the idle
                # SP/HWDGE queue (~1.2us less SWDGE generation each)
                if rb == RB - 1:
                    pieces = {3: 0, 5: 4, 7: 6}  # ri -> start row
                else:
                    pieces = {3: 0, 7: 4}
                if ri in pieces:
                    lo = pieces[ri]
                    if rb == RB - 1 and ri >= 5:
                        nc.sync.dma_start(out=y_blk[rb, :, lo:ri + 1],
                                          in_=y_sb[:, lo:ri + 1])
                    else:
                        nc.gpsimd.dma_start(out=y_blk[rb, :, lo:ri + 1],
                                            in_=y_sb[:, lo:ri + 1])
                    if ri == RB - 1:
                        y_sbs.pop(rb)

            # stage A leads by 9 rows: the v projections (independent of
            # attnT) keep the PE busy through the softmax-B -> attnT
            # transpose latency at phase-3 entry
            for r in range(R + 12):
                if r < R:
                    stage_a(r)
                # entry: i-tile-0/1 halves of rows 0-3 run first (they need
                # only the early attnT transposes), the 2/3 halves catch up
                # two per iteration once the late transposes land
                if r == 9:
                    stage_b(0, its=(0, 1), done=False)
                    stage_b(1, its=(0, 1), done=False)
                elif r == 10:
                    stage_b(2, its=(0, 1), done=False)
                    stage_b(3, its=(0, 1), done=False)
                elif r == 11:
                    stage_b(0, its=(2, 3))
                    stage_b(1, its=(2, 3))
                elif r == 12:
                    stage_b(2, its=(2, 3))
                    stage_b(3, its=(2, 3))
                elif 0 <= r - 9 < R:
                    stage_b(r - 9)
                if 0 <= r - 11 < R:
                    stage_t(r - 11)
                if 0 <= r - 12 < R:
                    stage_c(r - 12)

    return nc


def _get_program():
    if "nc" not in _CACHE:
        nc = build_program()
        nc.finalize()
        _CACHE["nc"] = nc
    return _CACHE["nc"]


def make_in_maps(x, Wq, Wkv, Wo):
    """Host-side sharding: core = bi*4 + hpi."""
    scale = (64.0 ** -0.5) * (64.0 ** -0.5)
    x = np.asarray(x, np.float32)
    Wq = np.asarray(Wq, np.float32) * np.float32(scale)
    Wkv = np.asarray(Wkv, np.float32)
    Wo = np.asarray(Wo, np.float32)
    b = x.shape[0] // R
    xT = np.ascontiguousarray(
        x.reshape(b, R, N, C).transpose(0, 1, 3, 2)).astype(np.float16)

    def swap_heads(w):  # [C, 128] -> column halves swapped
        return np.ascontiguousarray(
            np.concatenate([w[:, 64:], w[:, :64]], axis=1))

    in_maps = []
    for core in range(NCORES):
        bi, hpi = divmod(core, 4)
        cols = slice(hpi * HP, (hpi + 1) * HP)
        wq_c = np.ascontiguousarray(Wq[:, cols]).astype(np.float16)
        wk_c = np.ascontiguousarray(Wkv[:, cols]).astype(np.float16)
        in_maps.append({
            "xT": xT[bi],
            "idm": np.eye(128, dtype=np.float16),
            "wqe": wq_c,
            "wqo": swap_heads(wq_c),
            "wke": wk_c,
            "wko": swap_heads(wk_c),
            "wv": np.ascontiguousarray(
                Wkv[:, 512 + hpi * HP: 512 + (hpi + 1) * HP]).astype(np.float16),
            "wo": np.ascontiguousarray(Wo[cols, :]).astype(np.float16),
        })
    return in_maps


def combine_outputs(ys, bo):
    """ys: list of 8 [R, N, E] partials in core order; returns [B, n, dim]."""
    ys = [np.asarray(t, np.float32) for t in ys]
    y0 = ys[0] + ys[1] + ys[2] + ys[3]
    y1 = ys[4] + ys[5] + ys[6] + ys[7]
    y = np.concatenate([y0, y1], axis=0).reshape(2 * R, N, E)
    return (y + np.asarray(bo, np.float32)).astype(np.float32)


def kernel(x, Wq, Wkv, Wo, bo, tie_attn_dim):
    assert int(tie_attn_dim) == R, f"hardcoded for tie_attn_dim={R}"
    from concourse.bass_utils import run_bass_kernel_spmd

    nc = _get_program()
    in_maps = make_in_maps(x, Wq, Wkv, Wo)
    res = run_bass_kernel_spmd(nc, in_maps, list(range(NCORES)))
    ys = [res.results[c]["y"] for c in range(NCORES)]
    return combine_outputs(ys, bo)
